# revision 38
# baseline (speedup 1.0000x reference)
"""MultiHeadCrossAttention Trainium2 kernel (8 NeuronCores, SPMD).

Sharding: core c -> (n = c // 2, g = c % 2): one query batch n, half the
heads (8 of 16, embed slice g*512:(g+1)*512). Host compacts kv along KLEN
by the per-n mask (~50% survive), pads to KC = 128*T.

Key insight driving dtype choices: attention output is a softmax-weighted
AVERAGE of v, whose magnitude shrinks by the same sqrt(Neff) as the
weight-noise averaging gain -- so final rel err ~= per-weight RMS error.
fp8 weights (3-5% RMS) can never pass rel<2e-2; everything on the weight
and value paths stays bf16 (~0.2-1.8% RMS).

v4 changes over v3 (baseline 218us):
  - AV col-tiling: the two heads' AV matmuls run CONCURRENTLY in PE column
    groups 0-1 / 64-127 (M=64 each, tile_position auto-derived from the
    output AP base partition) -> AV PE time halves. PSUM has_written is
    per-element, so both heads share one av bank with a single start=True.
  - Softmax denominators move to dedicated 4-way col-tiled M=32 matmuls
    (2 k-tiles x 2 heads per N=512 slot). A zero-prefix window trick
    (lhsT = [0 x r, ind, 0 x ...]) routes each (c,qb) denominator to psum
    row 32*g + (c*4+qb) of ONE persistent bank, accumulated all kernel and
    evacuated once at the end.
  - Startup: input tiles split in half (q) / at col 1024 (k,v) so the
    first projections start after ~1.5MB lands instead of ~6MB; 4 DMA
    queues (sync/vector/scalar/gpsimd); Q-projections all run during the
    input-DMA window; K(c0)/V woven just-in-time.
  - Output: av [128,512] evac'd bf16 per (c,qb) and DMA'd immediately
    (no 65-row staging, half the evac instructions, no DMA tail bunching).
  - exp is split across ScalarE (native Exp ACTIVATE) and VectorE
    (one-op Schraudolph fast-exp int16 trick) by greedy busy-balance,
    as in v3.
"""

import math
import sys
from contextlib import ExitStack

import numpy as np

for _p in ("/opt/trn_rl_repo",):
    if _p not in sys.path:
        sys.path.insert(0, _p)

import ml_dtypes

import concourse.bass as bass  # noqa: F401  (import registers lowering deps)
import concourse.tile as tile
from concourse import bacc, mybir
from concourse.bass_utils import run_bass_kernel_spmd

BF16 = ml_dtypes.bfloat16

N, QLEN, KLEN = 4, 2048, 2048
QDIM = KVDIM = 512
EMBED, HEADS = 1024, 16
HEAD_DIM = 64
N_CORES = 8
SCALE = 1.0 / math.sqrt(HEAD_DIM)  # 1/8
# VectorE Schraudolph constants for bf16 bits (128/octave, bias 127):
#   i16 = round(e * (128/ln2)/8 + (127*128 - c))), c calibrated so the
#   piecewise-linear overestimate is mean-zero.
S1_DVE = (128.0 / math.log(2.0)) / 8.0
S2_DVE = 127.0 * 128.0 - 7.37

_cache: dict = {}
last_exec_time_ns = None
last_results = None


class _Balance:
    """Greedy ScalarE/VectorE assignment by simulated busy time."""

    def __init__(self):
        # ScalarE's input-DMA issues + exp table load mostly drain before
        # the first evacuation demand arrives (~12us in); only a small
        # residual backlog biases the early picks toward VectorE.
        self.t_act = 500.0
        self.t_dve = 300.0

    @staticmethod
    def cost_act(fd):
        return (215.0 + fd) / 1.2

    @staticmethod
    def cost_dve(fd):
        return (120.0 + fd) / 0.96

    def pick(self, fd):
        ca = self.cost_act(fd)
        cd = self.cost_dve(fd)
        if self.t_act + ca <= self.t_dve + cd:
            self.t_act += ca
            return "act"
        self.t_dve += cd
        return "dve"


def _build(T: int, ql: int = QLEN):
    KC = 128 * T
    KA = min(KC, 1024)     # first k/v column block (tile-aligned)
    KB = KC - KA           # remainder
    TA = KA // 128         # k-tiles in the A block
    NQB = ql // 512
    dt = mybir.dt
    nc = bacc.Bacc("TRN2", target_bir_lowering=False, debug=False)

    # all inputs host-packed partition-major ([128, ...] with large
    # contiguous per-partition runs) so each tensor ships as ONE DMA with
    # 8-16KB descriptors instead of 4 DMAs x 128 small row-descriptors
    qT_d = nc.dram_tensor("qt", [128, 2, 4, 1024], dt.bfloat16,
                          kind="ExternalInput").ap()
    kT_d = nc.dram_tensor("kt", [128, 4, KC], dt.bfloat16,
                          kind="ExternalInput").ap()
    vT_d = nc.dram_tensor("vt", [128, 4, KC], dt.bfloat16,
                          kind="ExternalInput").ap()
    wq_d = nc.dram_tensor("wq", [128, 4, 512], dt.bfloat16,
                          kind="ExternalInput").ap()
    wk_d = nc.dram_tensor("wk", [128, 4, 512], dt.bfloat16,
                          kind="ExternalInput").ap()
    wv_d = nc.dram_tensor("wv", [128, 4, 512], dt.bfloat16,
                          kind="ExternalInput").ap()
    # indicator window tiles: [128, T, 63], col 31 = valid-bit of token
    # t*128+p, cols elsewhere zero. lhsT window [31-r : 63-r] puts the
    # indicator in PE column r of a col group -> psum row 32g + r.
    vind_d = nc.dram_tensor("vind", [128, T * 63], dt.bfloat16,
                            kind="ExternalInput").ap()
    # rows c*128 + h*64 + d: unnormalized AV (bf16)
    out_d = nc.dram_tensor("out", [512, ql], dt.bfloat16, kind="ExternalOutput").ap()
    # denominators: row 16*(parity*2+h) + (c*4+qb), cols = q within block
    dd_d = nc.dram_tensor("dd", [64, 512], dt.float32, kind="ExternalOutput").ap()

    bal = _Balance()

    with tile.TileContext(nc) as tc:
        with ExitStack() as ctx:
            persist = ctx.enter_context(tc.tile_pool(name="persist", bufs=1))

            qTin = [persist.tile([128, 4, 1024], dt.bfloat16, tag=f"qTin{h}",
                                 name=f"qTin{h}") for h in range(2)]
            kTin = persist.tile([128, 4, KC], dt.bfloat16, tag="kTin", name="kTin")
            vTin = persist.tile([128, 4, KC], dt.bfloat16, tag="vTin", name="vTin")
            wsb = {
                nm: persist.tile([128, 4, 512], dt.bfloat16, tag=nm, name=nm)
                for nm in ("wq", "wk", "wv")
            }
            qflat = [persist.tile([128, ql], dt.bfloat16, tag=f"qf{c}", name=f"qf{c}") for c in range(4)]
            kTz = [persist.tile([128, KC], dt.bfloat16, tag=f"kz{c}", name=f"kz{c}") for c in range(4)]
            # v tiles: one per k-tile t (so late vchunk emissions never
            # create write-after-read edges on earlier tiles' AV reads);
            # layout [128, c, h, 64]; pad tokens have v=0
            vsbT = [persist.tile([128, 4, 2, 64], dt.bfloat16, tag=f"v{t}",
                                 name=f"v{t}") for t in range(T)]
            vind = persist.tile([128, T, 63], dt.bfloat16, tag="vind", name="vind")
            junk = persist.tile([128, 512], dt.bfloat16, tag="junk", name="junk")
            exp_warm = persist.tile([1, 8], dt.bfloat16, tag="ew", name="ew")

            # ---- input DMAs, one per tensor, on three issue queues in
            # need-order: q-path on sync, k-path on scalar, v-path on
            # gpsimd; the second q half rides the gpsimd tail ----
            nc.sync.dma_start(wsb["wq"], wq_d)
            nc.sync.dma_start(qTin[0], qT_d[:, 0])
            nc.scalar.dma_start(wsb["wk"], wk_d)
            nc.scalar.dma_start(kTin, kT_d)
            nc.gpsimd.dma_start(wsb["wv"], wv_d)
            nc.gpsimd.dma_start(vind, vind_d.rearrange("p (t w) -> p t w", t=T))
            nc.gpsimd.dma_start(vTin, vT_d)
            nc.gpsimd.dma_start(qTin[1], qT_d[:, 1])

            with tc.tile_pool(name="psE", bufs=3, space="PSUM") as psE, \
                 tc.tile_pool(name="psO", bufs=1, space="PSUM") as psO, \
                 tc.tile_pool(name="psD", bufs=1, space="PSUM") as psD, \
                 tc.tile_pool(name="wxp", bufs=15) as wxp, \
                 tc.tile_pool(name="sbo", bufs=4) as sbo, \
                 tc.tile_pool(name="sbd", bufs=1) as sbd:

                # preload the ACT exp table during the DMA window
                nc.vector.memset(junk, 1.0)
                nc.scalar.activation(exp_warm[:, :], junk[0:1, 0:8],
                                     mybir.ActivationFunctionType.Exp, scale=SCALE)

                # PE clock-gate warmup during the input-DMA window
                for _ in range(2):
                    ps = psE.tile([128, 1024], dt.float32, tag="u", name="u")
                    for r in range(11):
                        nc.tensor.matmul(ps[:, 0:512], lhsT=junk[:, :128], rhs=junk,
                                         start=(r == 0), stop=(r == 10))

                def evac(out_ap, in_ap, fd):
                    if bal.pick(fd) == "act":
                        nc.scalar.copy(out_ap, in_ap)
                    else:
                        nc.vector.tensor_copy(out_ap, in_ap)

                def qchunk(c, half):
                    def emit():
                        ps = psE.tile([128, 1024], dt.float32, tag="u", name="u")
                        for j in range(4):
                            for s2 in range(2):
                                nc.tensor.matmul(
                                    ps[:, s2 * 512:(s2 + 1) * 512],
                                    lhsT=wsb["wq"][:, j, c * 128:(c + 1) * 128],
                                    rhs=qTin[half][:, j, s2 * 512:(s2 + 1) * 512],
                                    start=(j == 0), stop=(j == 3),
                                )
                        evac(qflat[c][:, half * 1024:(half + 1) * 1024], ps[:, :], 1024)
                    return emit

                def kchunkA(c):
                    def emit():
                        ps = psE.tile([128, 1024], dt.float32, tag="u", name="u")
                        for j in range(4):
                            for s2 in range(KA // 512):
                                nc.tensor.matmul(
                                    ps[:, s2 * 512:(s2 + 1) * 512],
                                    lhsT=wsb["wk"][:, j, c * 128:(c + 1) * 128],
                                    rhs=kTin[:, j, s2 * 512:(s2 + 1) * 512],
                                    start=(j == 0), stop=(j == 3),
                                )
                        evac(kTz[c][:, 0:KA], ps[:, 0:KA], KA)
                    return emit

                def kchunkB(c):
                    def emit():
                        ps = psE.tile([128, 1024], dt.float32, tag="u", name="u")
                        for s0 in range(0, KB, 512):
                            cw = min(512, KB - s0)
                            for j in range(4):
                                nc.tensor.matmul(
                                    ps[:, s0:s0 + cw],
                                    lhsT=wsb["wk"][:, j, c * 128:(c + 1) * 128],
                                    rhs=kTin[:, j, KA + s0:KA + s0 + cw],
                                    start=(j == 0), stop=(j == 3),
                                )
                        evac(kTz[c][:, KA:KC], ps[:, 0:KB], KB)
                    return emit

                def vchunk(t0, nt):
                    """V projection for ALL head-pairs, nt k-tiles at once."""
                    def emit():
                        ps = psE.tile([128, 1024], dt.float32, tag="u", name="u")
                        for j in range(4):
                            for tt in range(nt):
                                t = t0 + tt
                                nc.tensor.matmul(
                                    ps[:, tt * 512:(tt + 1) * 512],
                                    lhsT=vTin[:, j, t * 128:(t + 1) * 128],
                                    rhs=wsb["wv"][:, j, 0:512],
                                    start=(j == 0), stop=(j == 3),
                                )
                        for tt in range(nt):
                            src = ps[:, tt * 512:(tt + 1) * 512].rearrange(
                                "p (c h d) -> p c h d", c=4, h=2)
                            evac(vsbT[t0 + tt][:, :, :, :], src, 512)
                    return emit

                # persistent denominator bank + its bookkeeping
                dv = psD.tile([128, 512], dt.float32, tag="dv", name="dv")
                # PSUM pending-zero marking is per-partition: each col
                # group's FIRST matmul needs start=True
                dv_first = {g: True for g in range(4)}
                last_even = T - 1 if (T - 1) % 2 == 0 else T - 2
                last_odd = T - 1 if (T - 1) % 2 == 1 else T - 2

                # software pipeline: AV of unit i is emitted after E of unit
                # i+6, so the PE never waits on the exp engines. The pend
                # limit swells to 9 when the head item is a block-first
                # (k==0) unit, giving the previous block's av evacuation
                # time to clear the (single-buffered) av bank.
                pend = []
                last_wx = {}

                def emit_denoms(c, qb, kk, wx):
                    r = c * 4 + qb
                    for h in range(2):
                        g = (kk % 2) * 2 + h
                        is_last = (c == 3 and qb == 3 and h == 1 and
                                   kk == (last_even if kk % 2 == 0 else last_odd))
                        nc.tensor.matmul(
                            dv[32 * g:32 * (g + 1), :],
                            lhsT=vind[:, kk, 31 - r:63 - r],
                            rhs=wx[:, h, :],
                            start=dv_first[g], stop=is_last,
                            skip_group_check=True,
                            tile_position=(0, 32 * g),
                        )
                        dv_first[g] = False

                def flush_some(nflush):
                    """Pop up to nflush pend items: all AV quads first (they
                    pipeline at ~240ns when adjacent), then the denominator
                    quads, then any final evac/DMA. AV runs as 4x M=32
                    col-tiled matmuls -- structurally identical to the
                    denominator quads, so AV<->DQ transitions also stay on
                    the fast same-shape path."""
                    dq, fin = [], []
                    for _ in range(nflush):
                        if not pend:
                            break
                        c, qb, k, av, wx = pend.pop(0)
                        for g in range(4):
                            h, p = g // 2, g % 2
                            nc.tensor.matmul(
                                av[32 * g:32 * (g + 1), :],
                                lhsT=vsbT[k][:, c, h, 32 * p:32 * (p + 1)],
                                rhs=wx[:, h, :],
                                start=(k == 0), stop=(k == T - 1),
                                skip_group_check=True,
                                tile_position=(0, 32 * g),
                            )
                        if k % 2 == 1:
                            dq += [(c, qb, k - 1, last_wx[(c, qb)]),
                                   (c, qb, k, wx)]
                        elif k == T - 1:
                            dq += [(c, qb, k, wx)]
                        last_wx[(c, qb)] = wx
                        if k == T - 1:
                            fin.append((c, qb, av))
                    for c, qb, kk, w in dq:
                        emit_denoms(c, qb, kk, w)
                    for c, qb, av in fin:
                        ot = sbo.tile([128, 512], dt.bfloat16, tag="ot", name="ot")
                        evac(ot[:, :], av[:, :], 512)
                        eng = nc.gpsimd if (c * 4 + qb) % 2 == 0 else nc.sync
                        eng.dma_start(
                            out_d[c * 128:(c + 1) * 128, qb * 512:qb * 512 + 512],
                            ot[:, :])

                def push_energy(c, qb, k, av):
                    q0 = qb * 512
                    u = psE.tile([128, 1024], dt.float32, tag="u", name="u")
                    nc.tensor.matmul(
                        u[:, 0:512],
                        lhsT=kTz[c][0:64, k * 128:(k + 1) * 128],
                        rhs=qflat[c][0:64, q0:q0 + 512],
                        start=True, stop=True)
                    nc.tensor.matmul(
                        u[:, 512:1024],
                        lhsT=kTz[c][64:128, k * 128:(k + 1) * 128],
                        rhs=qflat[c][64:128, q0:q0 + 512],
                        start=True, stop=True)
                    wx = wxp.tile([128, 2, 512], dt.bfloat16, tag="wx", name="wx")
                    src = u[:, :].rearrange("p (h q) -> p h q", h=2)
                    if bal.pick(1024) == "act":
                        nc.scalar.activation(
                            wx[:, :, :], src,
                            mybir.ActivationFunctionType.Exp, scale=SCALE)
                    else:
                        nc.vector.tensor_scalar(
                            wx[:, :, :].bitcast(dt.int16), src, S1_DVE, S2_DVE,
                            mybir.AluOpType.mult, mybir.AluOpType.add)
                    pend.append((c, qb, k, av, wx))

                def attention_block(c, qb, lag=7):
                    av = psO.tile([128, 512], dt.float32, tag="av", name="av")
                    # energies pushed in adjacent runs of 3 (same-shape
                    # groups pipeline at ~220ns vs ~330ns cross-shape)
                    for k2 in range(0, T, 3):
                        for k in range(k2, min(k2 + 3, T)):
                            push_energy(c, qb, k, av)
                        while len(pend) > (10 if pend[0][2] == 0 else lag):
                            flush_some(3)

                # ---- schedule ----
                # upfront (overlaps input DMA): all first-half q
                # projections, then K(c0), then the first v k-tiles.
                for c in range(4):
                    qchunk(c, 0)()
                kchunkA(0)()
                if KB:
                    kchunkB(0)()
                vchunk(0, min(2, T))()

                # per-block filler chunks (emitted at block start):
                # remaining v tiles ASAP, then just-in-time K(c+1) and the
                # second-half q projections.
                fillers = {}
                vlist = []
                t0 = 2
                while t0 < T:
                    nt = min(2, T - t0)
                    vlist.append(vchunk(t0, nt))
                    t0 += nt
                fillers[0] = vlist[:2]
                fillers[1] = vlist[2:] + [qchunk(0, 1)]
                fillers[2] = [qchunk(1, 1)]
                fillers[3] = [kchunkA(1)] + ([kchunkB(1)] if KB else [])
                fillers[4] = [qchunk(2, 1)]
                fillers[5] = [qchunk(3, 1)]
                fillers[6] = [kchunkA(2)]
                fillers[7] = [kchunkB(2)] if KB else []
                fillers[10] = [kchunkA(3)]
                fillers[11] = [kchunkB(3)] if KB else []

                for c in range(4):
                    for qb in range(NQB):
                        for ch in fillers.get(c * 4 + qb, []):
                            ch()
                        # final block drains its pipeline eagerly so the
                        # last evac+DMA chain starts ASAP (shorter tail)
                        attention_block(c, qb,
                                        lag=3 if (c, qb) == (3, NQB - 1) else 6)
                while pend:
                    flush_some(3)

                # final: evacuate + ship the denominator bank (only the 16
                # used rows per col group; 4 small DMAs split over 2 queues)
                dsb = sbd.tile([128, 512], dt.float32, tag="dsb", name="dsb")
                evac(dsb[:, :], dv[:, :], 512)
                for g in range(4):
                    eng = nc.sync if g % 2 == 0 else nc.gpsimd
                    eng.dma_start(dd_d[16 * g:16 * (g + 1), :],
                                  dsb[32 * g:32 * g + 16, :])

    nc.compile()
    return nc


def _prepare(queries, keys, values, mask):
    """Host-side sharding: transpose, compact kv by mask, indicator tiles."""
    m = np.asarray(mask).reshape(N, KLEN) != 0
    idx = [np.nonzero(m[n])[0] for n in range(N)]
    cnts = [len(i) for i in idx]
    T = max(2, (max(cnts) + 127) // 128)
    KC = 128 * T

    kT_full = np.ascontiguousarray(np.asarray(keys, np.float32)[0].T)
    vT_full = np.ascontiguousarray(np.asarray(values, np.float32)[0].T)
    q32 = np.asarray(queries, np.float32)

    qT_n, kT_n, vT_n, vind_n = [], [], [], []
    for n in range(N):
        kt = np.zeros((KVDIM, KC), np.float32)
        vt = np.zeros((KVDIM, KC), np.float32)
        kt[:, :cnts[n]] = kT_full[:, idx[n]]
        vt[:, :cnts[n]] = vT_full[:, idx[n]]
        # indicator window: [128, T, 63], col 31 holds validity of token
        # t*128 + p
        ind = (np.arange(KC) < cnts[n]).astype(np.float32).reshape(T, 128).T
        w = np.zeros((128, T, 63), np.float32)
        w[:, :, 31] = ind
        vind_n.append(np.ascontiguousarray(w.reshape(128, -1)).astype(BF16))
        # partition-major packing: [p, j, cols] (and [p, half, j, q] for q)
        # so every tensor is one DMA with multi-KB contiguous runs
        kT_n.append(np.ascontiguousarray(
            kt.reshape(4, 128, KC).transpose(1, 0, 2)).astype(BF16))
        vT_n.append(np.ascontiguousarray(
            vt.reshape(4, 128, KC).transpose(1, 0, 2)).astype(BF16))
        qt = np.ascontiguousarray(q32[n].T)  # [512, 2048]
        qT_n.append(np.ascontiguousarray(
            qt.reshape(4, 128, 2, 1024).transpose(1, 2, 0, 3)).astype(BF16))
    return T, qT_n, kT_n, vT_n, vind_n


def kernel(queries, keys, values, mask, Wq, Wk, Wv, _trace=False):
    global last_exec_time_ns, last_results
    T, qT_n, kT_n, vT_n, vind_n = _prepare(queries, keys, values, mask)

    w_g = {}
    for nm, W in (("wq", Wq), ("wk", Wk), ("wv", Wv)):
        W = np.asarray(W, np.float32)
        w_g[nm] = [np.ascontiguousarray(
            W[:, g * 512:(g + 1) * 512].reshape(4, 128, 512).transpose(1, 0, 2)
        ).astype(BF16) for g in range(2)]

    nc = _cache.get(T)
    if nc is None:
        nc = _cache.setdefault(T, _build(T))

    in_maps = []
    for core in range(N_CORES):
        n, g = core // 2, core % 2
        in_maps.append({
            "qt": qT_n[n], "kt": kT_n[n], "vt": vT_n[n],
            "wq": w_g["wq"][g], "wk": w_g["wk"][g], "wv": w_g["wv"][g],
            "vind": vind_n[n],
        })

    res = run_bass_kernel_spmd(nc, in_maps, core_ids=list(range(N_CORES)),
                               trace=bool(_trace))
    last_exec_time_ns = res.exec_time_ns
    last_results = res

    full = np.empty((N, QLEN, EMBED), np.float32)
    for core in range(N_CORES):
        n, g = core // 2, core % 2
        o = np.asarray(res.results[core]["out"], np.float32)   # [512, QLEN]
        dv = np.asarray(res.results[core]["dd"], np.float32)   # [64, 512]
        for c in range(4):
            for h in range(2):
                num = o[c * 128 + h * 64: c * 128 + (h + 1) * 64, :]  # [64, Q]
                e0 = g * 512 + (c * 2 + h) * 64
                for qb in range(QLEN // 512):
                    r = c * 4 + qb
                    d = dv[16 * h + r] + dv[16 * (2 + h) + r]         # [512]
                    blk = num[:, qb * 512:(qb + 1) * 512] / d[None, :]
                    full[n, qb * 512:(qb + 1) * 512, e0:e0 + 64] = blk.T
    return full


# revision 39
# speedup vs baseline: 1.0091x; 1.0091x over previous
"""MultiHeadCrossAttention Trainium2 kernel (8 NeuronCores, SPMD).

Sharding: core c -> (n = c // 2, g = c % 2): one query batch n, half the
heads (8 of 16, embed slice g*512:(g+1)*512). Host compacts kv along KLEN
by the per-n mask (~50% survive), pads to KC = 128*T.

Key insight driving dtype choices: attention output is a softmax-weighted
AVERAGE of v, whose magnitude shrinks by the same sqrt(Neff) as the
weight-noise averaging gain -- so final rel err ~= per-weight RMS error.
fp8 weights (3-5% RMS) can never pass rel<2e-2; everything on the weight
and value paths stays bf16 (~0.2-1.8% RMS).

v4 changes over v3 (baseline 218us):
  - AV col-tiling: the two heads' AV matmuls run CONCURRENTLY in PE column
    groups 0-1 / 64-127 (M=64 each, tile_position auto-derived from the
    output AP base partition) -> AV PE time halves. PSUM has_written is
    per-element, so both heads share one av bank with a single start=True.
  - Softmax denominators move to dedicated 4-way col-tiled M=32 matmuls
    (2 k-tiles x 2 heads per N=512 slot). A zero-prefix window trick
    (lhsT = [0 x r, ind, 0 x ...]) routes each (c,qb) denominator to psum
    row 32*g + (c*4+qb) of ONE persistent bank, accumulated all kernel and
    evacuated once at the end.
  - All attention matmul groups are uniform 4x M=32 col-tiled quads
    (AV + denominators) or adjacent same-shape runs (energy x3): same-
    shape back-to-back groups issue at ~216-227ns on the PE vs ~330ns
    for shape switches.
  - Startup: every input ships as ONE host-packed partition-major DMA
    (8-16KB contiguous runs per partition); first-half Q projections,
    K(c0) and the first V tiles run inside the input-DMA window.
  - Output: av [128,512] evac'd bf16 per (c,qb) and DMA'd immediately
    (no 65-row staging, half the evac instructions, no DMA tail bunching).
  - exp is split across ScalarE (native Exp ACTIVATE) and VectorE
    (one-op Schraudolph fast-exp int16 trick) by greedy busy-balance,
    as in v3.
"""

import math
import sys
from contextlib import ExitStack

import numpy as np

for _p in ("/opt/trn_rl_repo",):
    if _p not in sys.path:
        sys.path.insert(0, _p)

import ml_dtypes

import concourse.bass as bass  # noqa: F401  (import registers lowering deps)
import concourse.tile as tile
from concourse import bacc, mybir
from concourse.bass_utils import run_bass_kernel_spmd

BF16 = ml_dtypes.bfloat16

N, QLEN, KLEN = 4, 2048, 2048
QDIM = KVDIM = 512
EMBED, HEADS = 1024, 16
HEAD_DIM = 64
N_CORES = 8
SCALE = 1.0 / math.sqrt(HEAD_DIM)  # 1/8
# VectorE Schraudolph constants for bf16 bits (128/octave, bias 127):
#   i16 = round(e * (128/ln2)/8 + (127*128 - c))), c calibrated so the
#   piecewise-linear overestimate is mean-zero.
S1_DVE = (128.0 / math.log(2.0)) / 8.0
S2_DVE = 127.0 * 128.0 - 7.37

_cache: dict = {}
last_exec_time_ns = None
last_results = None


class _Balance:
    """Greedy ScalarE/VectorE assignment by simulated busy time."""

    def __init__(self):
        # ScalarE's input-DMA issues + exp table load mostly drain before
        # the first evacuation demand arrives (~12us in); only a small
        # residual backlog biases the early picks toward VectorE.
        self.t_act = 500.0
        self.t_dve = 300.0

    @staticmethod
    def cost_act(fd):
        return (215.0 + fd) / 1.2

    @staticmethod
    def cost_dve(fd):
        return (120.0 + fd) / 0.96

    def pick(self, fd):
        ca = self.cost_act(fd)
        cd = self.cost_dve(fd)
        if self.t_act + ca <= self.t_dve + cd:
            self.t_act += ca
            return "act"
        self.t_dve += cd
        return "dve"


def _build(T: int, ql: int = QLEN):
    KC = 128 * T
    KA = min(KC, 1024)     # first k/v column block (tile-aligned)
    KB = KC - KA           # remainder
    TA = KA // 128         # k-tiles in the A block
    NQB = ql // 512
    dt = mybir.dt
    nc = bacc.Bacc("TRN2", target_bir_lowering=False, debug=False)

    # all inputs host-packed partition-major ([128, ...] with large
    # contiguous per-partition runs) so each tensor ships as ONE DMA with
    # 8-16KB descriptors instead of 4 DMAs x 128 small row-descriptors
    qT_d = nc.dram_tensor("qt", [128, 2, 4, 1024], dt.bfloat16,
                          kind="ExternalInput").ap()
    kT_d = nc.dram_tensor("kt", [128, 4, KC], dt.bfloat16,
                          kind="ExternalInput").ap()
    vT_d = nc.dram_tensor("vt", [128, 4, KC], dt.bfloat16,
                          kind="ExternalInput").ap()
    wq_d = nc.dram_tensor("wq", [128, 4, 512], dt.bfloat16,
                          kind="ExternalInput").ap()
    wk_d = nc.dram_tensor("wk", [128, 4, 512], dt.bfloat16,
                          kind="ExternalInput").ap()
    wv_d = nc.dram_tensor("wv", [128, 4, 512], dt.bfloat16,
                          kind="ExternalInput").ap()
    # indicator window tiles: [128, T, 63], col 31 = valid-bit of token
    # t*128+p, cols elsewhere zero. lhsT window [31-r : 63-r] puts the
    # indicator in PE column r of a col group -> psum row 32g + r.
    vind_d = nc.dram_tensor("vind", [128, T * 63], dt.bfloat16,
                            kind="ExternalInput").ap()
    # rows c*128 + h*64 + d: unnormalized AV (bf16)
    out_d = nc.dram_tensor("out", [512, ql], dt.bfloat16, kind="ExternalOutput").ap()
    # denominators: row 16*(parity*2+h) + (c*4+qb), cols = q within block
    dd_d = nc.dram_tensor("dd", [64, 512], dt.float32, kind="ExternalOutput").ap()

    bal = _Balance()

    with tile.TileContext(nc) as tc:
        with ExitStack() as ctx:
            persist = ctx.enter_context(tc.tile_pool(name="persist", bufs=1))

            qTin = [persist.tile([128, 4, 1024], dt.bfloat16, tag=f"qTin{h}",
                                 name=f"qTin{h}") for h in range(2)]
            kTin = persist.tile([128, 4, KC], dt.bfloat16, tag="kTin", name="kTin")
            vTin = persist.tile([128, 4, KC], dt.bfloat16, tag="vTin", name="vTin")
            wsb = {
                nm: persist.tile([128, 4, 512], dt.bfloat16, tag=nm, name=nm)
                for nm in ("wq", "wk", "wv")
            }
            qflat = [persist.tile([128, ql], dt.bfloat16, tag=f"qf{c}", name=f"qf{c}") for c in range(4)]
            kTz = [persist.tile([128, KC], dt.bfloat16, tag=f"kz{c}", name=f"kz{c}") for c in range(4)]
            # v tiles: one per k-tile t (so late vchunk emissions never
            # create write-after-read edges on earlier tiles' AV reads);
            # layout [128, c, h, 64]; pad tokens have v=0
            vsbT = [persist.tile([128, 4, 2, 64], dt.bfloat16, tag=f"v{t}",
                                 name=f"v{t}") for t in range(T)]
            vind = persist.tile([128, T, 63], dt.bfloat16, tag="vind", name="vind")
            junk = persist.tile([128, 512], dt.bfloat16, tag="junk", name="junk")
            exp_warm = persist.tile([1, 8], dt.bfloat16, tag="ew", name="ew")

            # ---- input DMAs, one per tensor, on three issue queues in
            # need-order: q-path on sync, k-path on scalar, v-path on
            # gpsimd; the second q half rides the gpsimd tail ----
            nc.sync.dma_start(wsb["wq"], wq_d)
            nc.sync.dma_start(qTin[0], qT_d[:, 0])
            nc.scalar.dma_start(wsb["wk"], wk_d)
            nc.scalar.dma_start(kTin, kT_d)
            nc.gpsimd.dma_start(wsb["wv"], wv_d)
            nc.gpsimd.dma_start(vind, vind_d.rearrange("p (t w) -> p t w", t=T))
            nc.gpsimd.dma_start(vTin, vT_d)
            nc.gpsimd.dma_start(qTin[1], qT_d[:, 1])

            with tc.tile_pool(name="psE", bufs=3, space="PSUM") as psE, \
                 tc.tile_pool(name="psO", bufs=1, space="PSUM") as psO, \
                 tc.tile_pool(name="psD", bufs=1, space="PSUM") as psD, \
                 tc.tile_pool(name="wxp", bufs=15) as wxp, \
                 tc.tile_pool(name="sbo", bufs=4) as sbo, \
                 tc.tile_pool(name="sbd", bufs=1) as sbd:

                # preload the ACT exp table during the DMA window
                nc.vector.memset(junk, 1.0)
                nc.scalar.activation(exp_warm[:, :], junk[0:1, 0:8],
                                     mybir.ActivationFunctionType.Exp, scale=SCALE)

                # PE clock-gate warmup during the input-DMA window
                for _ in range(2):
                    ps = psE.tile([128, 1024], dt.float32, tag="u", name="u")
                    for r in range(11):
                        nc.tensor.matmul(ps[:, 0:512], lhsT=junk[:, :128], rhs=junk,
                                         start=(r == 0), stop=(r == 10))

                def evac(out_ap, in_ap, fd):
                    if bal.pick(fd) == "act":
                        nc.scalar.copy(out_ap, in_ap)
                    else:
                        nc.vector.tensor_copy(out_ap, in_ap)

                def qchunk(c, half):
                    def emit():
                        ps = psE.tile([128, 1024], dt.float32, tag="u", name="u")
                        for j in range(4):
                            for s2 in range(2):
                                nc.tensor.matmul(
                                    ps[:, s2 * 512:(s2 + 1) * 512],
                                    lhsT=wsb["wq"][:, j, c * 128:(c + 1) * 128],
                                    rhs=qTin[half][:, j, s2 * 512:(s2 + 1) * 512],
                                    start=(j == 0), stop=(j == 3),
                                )
                        evac(qflat[c][:, half * 1024:(half + 1) * 1024], ps[:, :], 1024)
                    return emit

                def kchunkA(c):
                    def emit():
                        ps = psE.tile([128, 1024], dt.float32, tag="u", name="u")
                        for j in range(4):
                            for s2 in range(KA // 512):
                                nc.tensor.matmul(
                                    ps[:, s2 * 512:(s2 + 1) * 512],
                                    lhsT=wsb["wk"][:, j, c * 128:(c + 1) * 128],
                                    rhs=kTin[:, j, s2 * 512:(s2 + 1) * 512],
                                    start=(j == 0), stop=(j == 3),
                                )
                        evac(kTz[c][:, 0:KA], ps[:, 0:KA], KA)
                    return emit

                def kchunkB(c):
                    def emit():
                        ps = psE.tile([128, 1024], dt.float32, tag="u", name="u")
                        for s0 in range(0, KB, 512):
                            cw = min(512, KB - s0)
                            for j in range(4):
                                nc.tensor.matmul(
                                    ps[:, s0:s0 + cw],
                                    lhsT=wsb["wk"][:, j, c * 128:(c + 1) * 128],
                                    rhs=kTin[:, j, KA + s0:KA + s0 + cw],
                                    start=(j == 0), stop=(j == 3),
                                )
                        evac(kTz[c][:, KA:KC], ps[:, 0:KB], KB)
                    return emit

                def vchunk(t0, nt):
                    """V projection for ALL head-pairs, nt k-tiles at once."""
                    def emit():
                        ps = psE.tile([128, 1024], dt.float32, tag="u", name="u")
                        for j in range(4):
                            for tt in range(nt):
                                t = t0 + tt
                                nc.tensor.matmul(
                                    ps[:, tt * 512:(tt + 1) * 512],
                                    lhsT=vTin[:, j, t * 128:(t + 1) * 128],
                                    rhs=wsb["wv"][:, j, 0:512],
                                    start=(j == 0), stop=(j == 3),
                                )
                        for tt in range(nt):
                            src = ps[:, tt * 512:(tt + 1) * 512].rearrange(
                                "p (c h d) -> p c h d", c=4, h=2)
                            evac(vsbT[t0 + tt][:, :, :, :], src, 512)
                    return emit

                # persistent denominator bank + its bookkeeping
                dv = psD.tile([128, 512], dt.float32, tag="dv", name="dv")
                # PSUM pending-zero marking is per-partition: each col
                # group's FIRST matmul needs start=True
                dv_first = {g: True for g in range(4)}
                last_even = T - 1 if (T - 1) % 2 == 0 else T - 2
                last_odd = T - 1 if (T - 1) % 2 == 1 else T - 2

                # software pipeline: AV of unit i is emitted after E of unit
                # i+6, so the PE never waits on the exp engines. The pend
                # limit swells to 9 when the head item is a block-first
                # (k==0) unit, giving the previous block's av evacuation
                # time to clear the (single-buffered) av bank.
                pend = []
                last_wx = {}

                def emit_denoms(c, qb, kk, wx):
                    r = c * 4 + qb
                    for h in range(2):
                        g = (kk % 2) * 2 + h
                        is_last = (c == 3 and qb == 3 and h == 1 and
                                   kk == (last_even if kk % 2 == 0 else last_odd))
                        nc.tensor.matmul(
                            dv[32 * g:32 * (g + 1), :],
                            lhsT=vind[:, kk, 31 - r:63 - r],
                            rhs=wx[:, h, :],
                            start=dv_first[g], stop=is_last,
                            skip_group_check=True,
                            tile_position=(0, 32 * g),
                        )
                        dv_first[g] = False

                def flush_some(nflush):
                    """Pop up to nflush pend items: all AV quads first (they
                    pipeline at ~240ns when adjacent), then the denominator
                    quads, then any final evac/DMA. AV runs as 4x M=32
                    col-tiled matmuls -- structurally identical to the
                    denominator quads, so AV<->DQ transitions also stay on
                    the fast same-shape path."""
                    dq, fin = [], []
                    for _ in range(nflush):
                        if not pend:
                            break
                        c, qb, k, av, wx = pend.pop(0)
                        for g in range(4):
                            h, p = g // 2, g % 2
                            nc.tensor.matmul(
                                av[32 * g:32 * (g + 1), :],
                                lhsT=vsbT[k][:, c, h, 32 * p:32 * (p + 1)],
                                rhs=wx[:, h, :],
                                start=(k == 0), stop=(k == T - 1),
                                skip_group_check=True,
                                tile_position=(0, 32 * g),
                            )
                        if k % 2 == 1:
                            dq += [(c, qb, k - 1, last_wx[(c, qb)]),
                                   (c, qb, k, wx)]
                        elif k == T - 1:
                            dq += [(c, qb, k, wx)]
                        last_wx[(c, qb)] = wx
                        if k == T - 1:
                            fin.append((c, qb, av))
                    for c, qb, kk, w in dq:
                        emit_denoms(c, qb, kk, w)
                    for c, qb, av in fin:
                        ot = sbo.tile([128, 512], dt.bfloat16, tag="ot", name="ot")
                        evac(ot[:, :], av[:, :], 512)
                        eng = nc.gpsimd if (c * 4 + qb) % 2 == 0 else nc.sync
                        eng.dma_start(
                            out_d[c * 128:(c + 1) * 128, qb * 512:qb * 512 + 512],
                            ot[:, :])

                def push_energy(c, qb, k, av):
                    q0 = qb * 512
                    u = psE.tile([128, 1024], dt.float32, tag="u", name="u")
                    nc.tensor.matmul(
                        u[:, 0:512],
                        lhsT=kTz[c][0:64, k * 128:(k + 1) * 128],
                        rhs=qflat[c][0:64, q0:q0 + 512],
                        start=True, stop=True)
                    nc.tensor.matmul(
                        u[:, 512:1024],
                        lhsT=kTz[c][64:128, k * 128:(k + 1) * 128],
                        rhs=qflat[c][64:128, q0:q0 + 512],
                        start=True, stop=True)
                    wx = wxp.tile([128, 2, 512], dt.bfloat16, tag="wx", name="wx")
                    src = u[:, :].rearrange("p (h q) -> p h q", h=2)
                    if bal.pick(1024) == "act":
                        nc.scalar.activation(
                            wx[:, :, :], src,
                            mybir.ActivationFunctionType.Exp, scale=SCALE)
                    else:
                        nc.vector.tensor_scalar(
                            wx[:, :, :].bitcast(dt.int16), src, S1_DVE, S2_DVE,
                            mybir.AluOpType.mult, mybir.AluOpType.add)
                    pend.append((c, qb, k, av, wx))

                def attention_block(c, qb, lag=6):
                    av = psO.tile([128, 512], dt.float32, tag="av", name="av")
                    # energies pushed in adjacent runs of 3 (same-shape
                    # groups pipeline at ~220ns vs ~330ns cross-shape)
                    for k2 in range(0, T, 3):
                        for k in range(k2, min(k2 + 3, T)):
                            push_energy(c, qb, k, av)
                        while len(pend) > (9 if pend[0][2] == 0 else lag):
                            flush_some(3)

                # ---- schedule ----
                # upfront (overlaps input DMA): all first-half q
                # projections, then K(c0), then the first v k-tiles.
                for c in range(4):
                    qchunk(c, 0)()
                kchunkA(0)()
                if KB:
                    kchunkB(0)()
                vchunk(0, min(2, T))()

                # per-block filler chunks (emitted at block start):
                # remaining v tiles ASAP, then just-in-time K(c+1) and the
                # second-half q projections.
                fillers = {}
                vlist = []
                t0 = 2
                while t0 < T:
                    nt = min(2, T - t0)
                    vlist.append(vchunk(t0, nt))
                    t0 += nt
                fillers[0] = vlist[:2]
                fillers[1] = vlist[2:] + [qchunk(0, 1)]
                fillers[2] = [qchunk(1, 1)]
                fillers[3] = [kchunkA(1)] + ([kchunkB(1)] if KB else [])
                fillers[4] = [qchunk(2, 1)]
                fillers[5] = [qchunk(3, 1)]
                fillers[6] = [kchunkA(2)]
                fillers[7] = [kchunkB(2)] if KB else []
                fillers[10] = [kchunkA(3)]
                fillers[11] = [kchunkB(3)] if KB else []

                for c in range(4):
                    for qb in range(NQB):
                        for ch in fillers.get(c * 4 + qb, []):
                            ch()
                        # final block drains its pipeline eagerly so the
                        # last evac+DMA chain starts ASAP (shorter tail)
                        attention_block(c, qb,
                                        lag=3 if (c, qb) == (3, NQB - 1) else 6)
                while pend:
                    flush_some(3)

                # final: evacuate + ship the denominator bank (only the 16
                # used rows per col group; 4 small DMAs split over 2 queues)
                dsb = sbd.tile([128, 512], dt.float32, tag="dsb", name="dsb")
                evac(dsb[:, :], dv[:, :], 512)
                for g in range(4):
                    eng = nc.sync if g % 2 == 0 else nc.gpsimd
                    eng.dma_start(dd_d[16 * g:16 * (g + 1), :],
                                  dsb[32 * g:32 * g + 16, :])

    nc.compile()
    return nc


def _prepare(queries, keys, values, mask):
    """Host-side sharding: transpose, compact kv by mask, indicator tiles."""
    m = np.asarray(mask).reshape(N, KLEN) != 0
    idx = [np.nonzero(m[n])[0] for n in range(N)]
    cnts = [len(i) for i in idx]
    T = max(2, (max(cnts) + 127) // 128)
    KC = 128 * T

    kT_full = np.ascontiguousarray(np.asarray(keys, np.float32)[0].T)
    vT_full = np.ascontiguousarray(np.asarray(values, np.float32)[0].T)
    q32 = np.asarray(queries, np.float32)

    qT_n, kT_n, vT_n, vind_n = [], [], [], []
    for n in range(N):
        kt = np.zeros((KVDIM, KC), np.float32)
        vt = np.zeros((KVDIM, KC), np.float32)
        kt[:, :cnts[n]] = kT_full[:, idx[n]]
        vt[:, :cnts[n]] = vT_full[:, idx[n]]
        # indicator window: [128, T, 63], col 31 holds validity of token
        # t*128 + p
        ind = (np.arange(KC) < cnts[n]).astype(np.float32).reshape(T, 128).T
        w = np.zeros((128, T, 63), np.float32)
        w[:, :, 31] = ind
        vind_n.append(np.ascontiguousarray(w.reshape(128, -1)).astype(BF16))
        # partition-major packing: [p, j, cols] (and [p, half, j, q] for q)
        # so every tensor is one DMA with multi-KB contiguous runs
        kT_n.append(np.ascontiguousarray(
            kt.reshape(4, 128, KC).transpose(1, 0, 2)).astype(BF16))
        vT_n.append(np.ascontiguousarray(
            vt.reshape(4, 128, KC).transpose(1, 0, 2)).astype(BF16))
        qt = np.ascontiguousarray(q32[n].T)  # [512, 2048]
        qT_n.append(np.ascontiguousarray(
            qt.reshape(4, 128, 2, 1024).transpose(1, 2, 0, 3)).astype(BF16))
    return T, qT_n, kT_n, vT_n, vind_n


def kernel(queries, keys, values, mask, Wq, Wk, Wv, _trace=False):
    global last_exec_time_ns, last_results
    T, qT_n, kT_n, vT_n, vind_n = _prepare(queries, keys, values, mask)

    w_g = {}
    for nm, W in (("wq", Wq), ("wk", Wk), ("wv", Wv)):
        W = np.asarray(W, np.float32)
        w_g[nm] = [np.ascontiguousarray(
            W[:, g * 512:(g + 1) * 512].reshape(4, 128, 512).transpose(1, 0, 2)
        ).astype(BF16) for g in range(2)]

    nc = _cache.get(T)
    if nc is None:
        nc = _cache.setdefault(T, _build(T))

    in_maps = []
    for core in range(N_CORES):
        n, g = core // 2, core % 2
        in_maps.append({
            "qt": qT_n[n], "kt": kT_n[n], "vt": vT_n[n],
            "wq": w_g["wq"][g], "wk": w_g["wk"][g], "wv": w_g["wv"][g],
            "vind": vind_n[n],
        })

    res = run_bass_kernel_spmd(nc, in_maps, core_ids=list(range(N_CORES)),
                               trace=bool(_trace))
    last_exec_time_ns = res.exec_time_ns
    last_results = res

    full = np.empty((N, QLEN, EMBED), np.float32)
    for core in range(N_CORES):
        n, g = core // 2, core % 2
        o = np.asarray(res.results[core]["out"], np.float32)   # [512, QLEN]
        dv = np.asarray(res.results[core]["dd"], np.float32)   # [64, 512]
        for c in range(4):
            for h in range(2):
                num = o[c * 128 + h * 64: c * 128 + (h + 1) * 64, :]  # [64, Q]
                e0 = g * 512 + (c * 2 + h) * 64
                for qb in range(QLEN // 512):
                    r = c * 4 + qb
                    d = dv[16 * h + r] + dv[16 * (2 + h) + r]         # [512]
                    blk = num[:, qb * 512:(qb + 1) * 512] / d[None, :]
                    full[n, qb * 512:(qb + 1) * 512, e0:e0 + 64] = blk.T
    return full
